# revision 9
# baseline (speedup 1.0000x reference)
"""
Trainium2 Bass kernel for nn_Attention_335007449901 (sparse window attention).

Model (per image, eval mode):
  q = BN(conv1x1(x, wq)); k = BN(conv1x1(x, wk)); v = BN(conv1x1(x, wv))
  windows of 7x7 over the 112x112 image -> T=256 window tokens,
  token features = (channel, within-window position p) pairs.
  dots[i,j] = sum_{c,p} q[c,pix(i,p)] k[c,pix(j,p)] * 0.125
  attn = softmax_j(dots); out[i,(c,p)] = sum_j attn[i,j] v[c,pix(j,p)]
  y = gelu(out); z = BN(conv1x1(y, wo) + bo); out = gelu(z + x)

Sharding: pure data parallel over batch, 4 images per core on 8 cores.

Key implementation ideas:
  * All BatchNorms are inference-affine -> folded into conv weights on the
    host. SCALE folded into wq. k's bias drops (softmax shift invariance
    along the normalization axis). v's bias commutes through the attention
    average (softmax rows sum to 1) and becomes a per-channel bias applied
    in the gelu. The final conv bias + BN fold into the last gelu's bias.
  * dots are computed TRANSPOSED (dots_T[j,i]) so the softmax normalization
    axis j lands on partitions; sums over j are rank-reduce matmuls with a
    ones vector; no max-subtraction is needed (|dots| <~ 30, exp is safe in
    fp32).
  * Window gathers are pure strided access patterns into the resident x
    image in SBUF - no data movement for windowing at all.
  * Per within-window position p (49 of them): q_p,k_p = [c,t] conv tiles;
    v_p = [t,c] conv tiles (computed transposed by swapping matmul
    operands); attention-weighted values then come out as [(c,p), t] which
    is exactly the layout the final 1x1 conv wants as its moving operand.
"""

import numpy as np

IN_C = 128
HIDE_C = 256
HC2 = 128
OUT_C = 128
WS = 7
SCALE = 0.125
EPS = 1e-5
B, H, W = 32, 112, 112
HW = H * W          # 12544
H1 = H // WS        # 16
W1 = W // WS        # 16
T = H1 * W1         # 256 windows
NCORES = 8
BPC = B // NCORES   # images per core

F32 = np.float32


def _pgroups():
    """Groups of 1-2 within-window positions with a uniform pixel-offset
    stride between members (so each group is one strided access pattern).
    Position p=(ws1,ws2) sits at pixel offset ws1*112+ws2 in its window cell.
    49 positions -> 24 pairs + 1 singleton."""
    groups = []
    for ws1 in range(WS):
        for b2 in range(3):
            groups.append(((ws1, 2 * b2), (ws1, 2 * b2 + 1), 1))
    for a in range(3):
        groups.append(((2 * a, 6), (2 * a + 1, 6), 112))
    groups.append(((6, 6), None, 0))
    return groups


def build_bass_kernel(bpc=BPC):
    import concourse.bass as bass
    import concourse.tile as tile
    import concourse.mybir as mybir
    from concourse import bacc

    f32 = mybir.dt.float32
    AF = mybir.ActivationFunctionType

    nc = bacc.Bacc("TRN2", target_bir_lowering=False)

    x_d = nc.dram_tensor("x", [bpc, IN_C, HW], f32, kind="ExternalInput")
    wqT_d = nc.dram_tensor("wqT", [IN_C, HC2], f32, kind="ExternalInput")
    wkT_d = nc.dram_tensor("wkT", [IN_C, HC2], f32, kind="ExternalInput")
    wvT_d = nc.dram_tensor("wvT", [IN_C, HIDE_C], f32, kind="ExternalInput")
    woT_d = nc.dram_tensor("woT", [HIDE_C, OUT_C], f32, kind="ExternalInput")
    # packed per-partition bias columns: [Bq, Bv_lo, Bv_hi, Bo]
    bias_d = nc.dram_tensor("biases", [128, 4], f32, kind="ExternalInput")
    out_d = nc.dram_tensor("out", [bpc, OUT_C, HW], f32, kind="ExternalOutput")

    groups = _pgroups()

    with tile.TileContext(nc) as tc:
        with (
            tc.tile_pool(name="singles", bufs=1) as singles,
            tc.tile_pool(name="xpool", bufs=2) as xpool,
            tc.tile_pool(name="opool", bufs=1) as opool,
            tc.tile_pool(name="qk_sb", bufs=3) as qk_sb,
            tc.tile_pool(name="v_sb", bufs=3) as v_sb_pool,
            tc.tile_pool(name="g_sb", bufs=2) as g_sb_pool,
            tc.tile_pool(name="attn_sb", bufs=2) as attn_pool,
            tc.tile_pool(name="tmp_sb", bufs=2) as tmp_pool,
            tc.tile_pool(name="xg_sb", bufs=3) as xg_pool,
            tc.tile_pool(name="small_sb", bufs=2) as small_pool,
            tc.tile_pool(name="ps_work", bufs=2, space="PSUM") as ps_work,
            tc.tile_pool(name="ps_dots", bufs=2, space="PSUM") as ps_dots,
            tc.tile_pool(name="ps_av", bufs=2, space="PSUM") as ps_av,
            tc.tile_pool(name="ps_o", bufs=2, space="PSUM") as ps_o_pool,
        ):
            # ---- weights / constants (resident) ----
            wqT = singles.tile([128, HC2], f32)
            nc.sync.dma_start(out=wqT, in_=wqT_d.ap())
            wkT = singles.tile([128, HC2], f32)
            nc.sync.dma_start(out=wkT, in_=wkT_d.ap())
            wvT = singles.tile([128, HIDE_C], f32)
            nc.sync.dma_start(out=wvT, in_=wvT_d.ap())
            woT = singles.tile([128, 2, OUT_C], f32)
            nc.sync.dma_start(
                out=woT, in_=woT_d.ap().rearrange("(kc p) m -> p kc m", kc=2)
            )
            biases = singles.tile([128, 4], f32)
            nc.sync.dma_start(out=biases, in_=bias_d.ap())
            bq_ap = biases[:, 0:1]
            bv_ap = [biases[:, 1:2], biases[:, 2:3]]
            bo_ap = biases[:, 3:4]

            ones_p = singles.tile([128, 1], f32)
            nc.vector.memset(ones_p, 1.0)
            ones_r = singles.tile([1, 128], f32)
            nc.vector.memset(ones_r, 1.0)

            for img in range(bpc):
                # ---- load x image ----
                x_img = xpool.tile([128, HW], f32, tag="ximg")
                nc.sync.dma_start(out=x_img, in_=x_d.ap()[img])
                x5 = x_img.rearrange("p (h a w b) -> p h a w b", h=H1, a=WS, b=WS)

                out_img = opool.tile([128, HW], f32, tag="oimg")
                o5 = out_img.rearrange("p (h a w b) -> p h a w b", h=H1, a=WS, b=WS)

                def grp_ap(t5, g):
                    """strided AP covering this group's positions: [p, cnt*T]"""
                    (ws1, ws2), p2, stride = g
                    if p2 is None:
                        return t5[:, :, ws1, :, ws2]
                    if stride == 1:
                        return t5[:, :, ws1, :, ws2:ws2 + 2].rearrange(
                            "p h w b -> p b h w"
                        )
                    return t5[:, :, ws1:ws1 + 2, :, ws2].rearrange(
                        "p h a w -> p a h w"
                    )

                # ---- phase 1: dots_T accumulation over the 49 positions ----
                dots = [ps_dots.tile([128, T], f32, tag="dots", name=f"dots{jc}")
                        for jc in (0, 1)]
                for gi, g in enumerate(groups):
                    cnt = 1 if g[1] is None else 2
                    N = cnt * T
                    rhs_x = grp_ap(x5, g)

                    q_ps = ps_work.tile([128, 512], f32, tag="pwork")
                    nc.tensor.matmul(q_ps[:, :N], lhsT=wqT, rhs=rhs_x,
                                     start=True, stop=True)
                    q_sbt = qk_sb.tile([128, 512], f32, tag="q")
                    nc.scalar.activation(q_sbt[:, :N], q_ps[:, :N],
                                         AF.Identity, bias=bq_ap, scale=1.0)

                    k_ps = ps_work.tile([128, 512], f32, tag="pwork")
                    nc.tensor.matmul(k_ps[:, :N], lhsT=wkT, rhs=rhs_x,
                                     start=True, stop=True)
                    k_sbt = qk_sb.tile([128, 512], f32, tag="k")
                    nc.vector.tensor_copy(k_sbt[:, :N], k_ps[:, :N])

                    first = gi == 0
                    last = gi == len(groups) - 1
                    for pi in range(cnt):
                        for jh in (0, 1):
                            nc.tensor.matmul(
                                dots[jh],
                                lhsT=k_sbt[:, pi * T + jh * 128: pi * T + jh * 128 + 128],
                                rhs=q_sbt[:, pi * T:(pi + 1) * T],
                                start=first and pi == 0,
                                stop=last and pi == cnt - 1,
                            )

                # ---- softmax over j (= partitions of dots_T) ----
                attn = [attn_pool.tile([128, T], f32, tag=f"attn{jc}", name=f"attn{jc}")
                        for jc in (0, 1)]
                for jc in (0, 1):
                    nc.scalar.activation(attn[jc], dots[jc], AF.Exp)
                s_ps = ps_dots.tile([1, T], f32, tag="dots", name="ssum")
                for jc in (0, 1):
                    nc.tensor.matmul(s_ps, lhsT=ones_p, rhs=attn[jc],
                                     start=jc == 0, stop=jc == 1)
                r_sb = small_pool.tile([1, T], f32, tag="rsb")
                nc.vector.reciprocal(r_sb, s_ps)
                rb_ps = ps_dots.tile([128, T], f32, tag="dots", name="rbcast")
                nc.tensor.matmul(rb_ps, lhsT=ones_r, rhs=r_sb, start=True, stop=True)
                for jc in (0, 1):
                    nc.vector.tensor_mul(attn[jc], attn[jc], rb_ps)

                # ---- phase 2: v-conv, attention-average, out-conv, residual ----
                for g in groups:
                    (ws1a, ws2a), p2, stride = g
                    cnt = 1 if p2 is None else 2
                    N = cnt * T
                    plist = [(ws1a, ws2a)] + ([p2] if p2 is not None else [])

                    # contiguous copy of this group's window columns of x
                    # (matmul stationary operands need single-free-dim APs,
                    # and it makes the residual read contiguous too)
                    xg = xg_pool.tile([128, 512], f32, tag="xg")
                    nc.gpsimd.tensor_copy(xg[:, :N], grp_ap(x5, g))

                    vsb = []
                    for pi, (ws1, ws2) in enumerate(plist):
                        v_ps = ps_work.tile([128, 512], f32, tag="pwork")
                        for jc in (0, 1):
                            nc.tensor.matmul(
                                v_ps[:, jc * HIDE_C:(jc + 1) * HIDE_C],
                                lhsT=xg[:, pi * T + jc * 128: pi * T + jc * 128 + 128],
                                rhs=wvT,
                                start=True, stop=True,
                            )
                        v_sbt = v_sb_pool.tile([128, 512], f32, tag="v")
                        nc.vector.tensor_copy(v_sbt, v_ps)
                        vsb.append(v_sbt)

                    g_tiles = []
                    for kc in (0, 1):
                        av = ps_av.tile([128, 512], f32, tag="av", name=f"av{kc}")
                        for pi in range(cnt):
                            for jc in (0, 1):
                                nc.tensor.matmul(
                                    av[:, pi * T:(pi + 1) * T],
                                    lhsT=vsb[pi][:, jc * HIDE_C + kc * 128:
                                                  jc * HIDE_C + kc * 128 + 128],
                                    rhs=attn[jc],
                                    start=jc == 0, stop=jc == 1,
                                )
                        g_t = g_sb_pool.tile([128, 512], f32, tag=f"g{kc}")
                        nc.scalar.activation(g_t[:, :N], av[:, :N], AF.Gelu,
                                             bias=bv_ap[kc], scale=1.0)
                        g_tiles.append(g_t)

                    o_ps = ps_o_pool.tile([128, 512], f32, tag="ops")
                    for pi in range(cnt):
                        for kc in (0, 1):
                            nc.tensor.matmul(
                                o_ps[:, pi * T:(pi + 1) * T],
                                lhsT=woT[:, kc, :],
                                rhs=g_tiles[kc][:, pi * T:(pi + 1) * T],
                                start=kc == 0, stop=kc == 1,
                            )
                    tmp = tmp_pool.tile([128, 512], f32, tag="tmp")
                    nc.vector.tensor_add(tmp[:, :N], o_ps[:, :N], xg[:, :N])
                    nc.scalar.activation(grp_ap(o5, g), tmp[:, :N], AF.Gelu,
                                         bias=bo_ap, scale=1.0)

                # ---- store ----
                nc.sync.dma_start(out=out_d.ap()[img], in_=out_img)

    nc.compile()
    return nc


def fold_params(wq, gq, bq, mq, vq, wk, gk, bk, mk, vk,
                wv, gv, bv, mv, vv, wo, bo, go, bbo, mo, vo):
    """Host-side BN/bias folding. Returns transposed folded weights and the
    packed per-partition bias columns."""
    aq = gq / np.sqrt(vq + EPS)
    wq_f = (SCALE * aq)[:, None] * wq
    Bq = SCALE * (bq - aq * mq)

    ak = gk / np.sqrt(vk + EPS)
    wk_f = ak[:, None] * wk          # k bias drops (softmax shift invariance)

    av = gv / np.sqrt(vv + EPS)
    wv_f = av[:, None] * wv
    Bv = bv - av * mv                # applied inside the first gelu

    ao = go / np.sqrt(vo + EPS)
    wo_f = ao[:, None] * wo
    Bo = ao * (bo - mo) + bbo        # conv bias + BN fold, inside last gelu

    biases = np.stack([Bq, Bv[:128], Bv[128:], Bo], axis=1).astype(F32)
    return (np.ascontiguousarray(wq_f.T.astype(F32)),
            np.ascontiguousarray(wk_f.T.astype(F32)),
            np.ascontiguousarray(wv_f.T.astype(F32)),
            np.ascontiguousarray(wo_f.T.astype(F32)),
            biases)


_CACHED = {}


def _get_nc(bpc=BPC):
    if bpc not in _CACHED:
        _CACHED[bpc] = build_bass_kernel(bpc)
    return _CACHED[bpc]


def kernel(**inputs):
    from concourse.bass_utils import run_bass_kernel_spmd

    x = np.asarray(inputs["x"], F32)
    wqT, wkT, wvT, woT, biases = fold_params(
        *[np.asarray(inputs[k], F32) for k in
          ("wq", "gq", "bq", "mq", "vq", "wk", "gk", "bk", "mk", "vk",
           "wv", "gv", "bv", "mv", "vv", "wo", "bo", "go", "bbo", "mo", "vo")]
    )

    nc = _get_nc(BPC)
    in_maps = []
    for c in range(NCORES):
        xs = np.ascontiguousarray(
            x[c * BPC:(c + 1) * BPC].reshape(BPC, IN_C, HW))
        in_maps.append({"x": xs, "wqT": wqT, "wkT": wkT, "wvT": wvT,
                        "woT": woT, "biases": biases})

    res = run_bass_kernel_spmd(nc, in_maps, list(range(NCORES)))
    outs = [res.results[c]["out"].reshape(BPC, OUT_C, H, W)
            for c in range(NCORES)]
    return np.concatenate(outs, axis=0)


# revision 11
# speedup vs baseline: 2.8920x; 2.8920x over previous
"""
Trainium2 Bass kernel for nn_Attention_335007449901 (sparse window attention).

Model (per image, eval mode):
  q = BN(conv1x1(x, wq)); k = BN(conv1x1(x, wk)); v = BN(conv1x1(x, wv))
  7x7 windows over the 112x112 image -> T=256 window tokens, token
  features = (channel, within-window position p) pairs (dim 128*49 / 256*49).
  dots[i,j] = <q_i, k_j> * 0.125 ; attn = softmax_j ; out = attn @ v
  y = gelu(out); z = BN(conv1x1(y, wo) + bo); out = gelu(z + x)

Sharding: pure data parallel over batch, 4 images per core on 8 cores.

Implementation notes:
  * BatchNorms folded into conv weights on the host; SCALE folded into wq;
    k's bias drops (softmax shift invariance along the normalized axis);
    v's bias passes through the attention average (rows sum to 1) into the
    first gelu's bias; final conv bias + BN fold into the last gelu's bias.
  * All matmul operands are bf16 (fp32 PSUM accumulation). fp32 matmuls on
    trn2 run as LOW/HIGH double passes - bf16 is 2x the throughput - and
    strided moving operands stream ~5x slower than contiguous ones, so a
    window-permuted bf16 copy of x (x_winb, built by the otherwise idle
    GPSIMD engine) provides contiguous operands for every matmul.
  * dots are computed TRANSPOSED (dots_T[j,i]) so softmax normalization is
    a ones-vector matmul reduce; no max subtraction needed (|dots| < ~30).
  * The residual add reads the original fp32 x image; the final gelu writes
    its output IN PLACE into the x image (each window position's columns
    are dead after their residual read), which saves a whole image buffer.
"""

import numpy as np

IN_C = 128
HIDE_C = 256
HC2 = 128
OUT_C = 128
WS = 7
SCALE = 0.125
EPS = 1e-5
B, H, W = 32, 112, 112
HW = H * W          # 12544
H1 = H // WS        # 16
W1 = W // WS        # 16
T = H1 * W1         # 256 windows
NP = WS * WS        # 49 positions
NCORES = 8
BPC = B // NCORES   # images per core

F32 = np.float32


def _pgroups():
    """Groups of 1-2 within-window positions with a uniform pixel-offset
    stride between members (so each group is one strided access pattern in
    the image layout). 49 positions -> 24 pairs + 1 singleton. Also carries
    each group's column base in the grouped window layout x_winb."""
    groups = []
    base = 0
    for ws1 in range(WS):
        for b2 in range(3):
            groups.append(((ws1, 2 * b2), (ws1, 2 * b2 + 1), base))
            base += 2 * T
    for a in range(3):
        groups.append(((2 * a, 6), (2 * a + 1, 6), base))
        base += 2 * T
    groups.append(((6, 6), None, base))
    return groups


def build_bass_kernel(bpc=BPC):
    import concourse.bass as bass
    import concourse.tile as tile
    import concourse.mybir as mybir
    from concourse import bacc

    f32 = mybir.dt.float32
    bf16 = mybir.dt.bfloat16
    AF = mybir.ActivationFunctionType

    nc = bacc.Bacc("TRN2", target_bir_lowering=False)

    x_d = nc.dram_tensor("x", [bpc, IN_C, HW], f32, kind="ExternalInput")
    wqT_d = nc.dram_tensor("wqT", [IN_C, HC2], bf16, kind="ExternalInput")
    wkT_d = nc.dram_tensor("wkT", [IN_C, HC2], bf16, kind="ExternalInput")
    wvT_d = nc.dram_tensor("wvT", [IN_C, HIDE_C], bf16, kind="ExternalInput")
    woT_d = nc.dram_tensor("woT", [HIDE_C, OUT_C], bf16, kind="ExternalInput")
    # packed per-partition bias columns: [Bq, Bv_lo, Bv_hi, Bo]
    bias_d = nc.dram_tensor("biases", [128, 4], f32, kind="ExternalInput")
    out_d = nc.dram_tensor("out", [bpc, OUT_C, HW], f32, kind="ExternalOutput")

    groups = _pgroups()

    with tile.TileContext(nc) as tc:
        with (
            tc.tile_pool(name="singles", bufs=1) as singles,
            tc.tile_pool(name="xpool", bufs=2) as xpool,
            tc.tile_pool(name="xwin", bufs=2) as xwin_pool,
            tc.tile_pool(name="qk_sb", bufs=3) as qk_sb,
            tc.tile_pool(name="v_sb", bufs=3) as v_sb_pool,
            tc.tile_pool(name="g_sb", bufs=2) as g_sb_pool,
            tc.tile_pool(name="attn_sb", bufs=2) as attn_pool,
            tc.tile_pool(name="tmp_sb", bufs=2) as tmp_pool,
            tc.tile_pool(name="small_sb", bufs=2) as small_pool,
            tc.tile_pool(name="ps_work", bufs=2, space="PSUM") as ps_work,
            tc.tile_pool(name="ps_dots", bufs=2, space="PSUM") as ps_dots,
            tc.tile_pool(name="ps_av", bufs=2, space="PSUM") as ps_av,
            tc.tile_pool(name="ps_o", bufs=2, space="PSUM") as ps_o_pool,
        ):
            # ---- weights / constants (resident) ----
            wqT = singles.tile([128, HC2], bf16)
            nc.sync.dma_start(out=wqT, in_=wqT_d.ap())
            wkT = singles.tile([128, HC2], bf16)
            nc.sync.dma_start(out=wkT, in_=wkT_d.ap())
            wvT = singles.tile([128, HIDE_C], bf16)
            nc.sync.dma_start(out=wvT, in_=wvT_d.ap())
            woT = singles.tile([128, 2, OUT_C], bf16)
            nc.sync.dma_start(
                out=woT, in_=woT_d.ap().rearrange("(kc p) m -> p kc m", kc=2)
            )
            biases = singles.tile([128, 4], f32)
            nc.sync.dma_start(out=biases, in_=bias_d.ap())
            bq_ap = biases[:, 0:1]
            bv_ap = [biases[:, 1:2], biases[:, 2:3]]
            bo_ap = biases[:, 3:4]

            ones_pb = singles.tile([128, 1], bf16)
            nc.vector.memset(ones_pb, 1.0)
            ones_r = singles.tile([1, 128], f32)
            nc.vector.memset(ones_r, 1.0)

            for img in range(bpc):
                # ---- load x image; build grouped bf16 window copy ----
                x_img = xpool.tile([128, HW], f32, tag="ximg")
                nc.sync.dma_start(out=x_img, in_=x_d.ap()[img])
                x5 = x_img.rearrange("p (h a w b) -> p h a w b", h=H1, a=WS, b=WS)

                def grp_ap(g):
                    """strided image-layout AP of this group's positions"""
                    (ws1, ws2), p2, _ = g
                    if p2 is None:
                        return x5[:, :, ws1, :, ws2]
                    if p2[0] == ws1:  # within-row pair, pixel stride 1
                        return x5[:, :, ws1, :, ws2:ws2 + 2].rearrange(
                            "p h w b -> p b h w")
                    return x5[:, :, ws1:ws1 + 2, :, ws2].rearrange(
                        "p h a w -> p a h w")

                x_winb = xwin_pool.tile([128, NP * T], bf16, tag="xwin")
                for g in groups:
                    N = T if g[1] is None else 2 * T
                    nc.gpsimd.tensor_copy(
                        x_winb[:, g[2]:g[2] + N], grp_ap(g))

                # ---- phase 1: dots_T accumulation over the 49 positions ----
                dots = [ps_dots.tile([128, T], f32, tag="dots", name=f"dots{jc}")
                        for jc in (0, 1)]
                for gi, g in enumerate(groups):
                    cnt = 1 if g[1] is None else 2
                    N = cnt * T
                    xb = x_winb[:, g[2]:g[2] + N]

                    q_ps = ps_work.tile([128, 512], f32, tag="pwork")
                    nc.tensor.matmul(q_ps[:, :N], lhsT=wqT, rhs=xb,
                                     start=True, stop=True)
                    q_sbt = qk_sb.tile([128, 512], bf16, tag="q")
                    nc.scalar.activation(q_sbt[:, :N], q_ps[:, :N],
                                         AF.Identity, bias=bq_ap, scale=1.0)

                    k_ps = ps_work.tile([128, 512], f32, tag="pwork")
                    nc.tensor.matmul(k_ps[:, :N], lhsT=wkT, rhs=xb,
                                     start=True, stop=True)
                    k_sbt = qk_sb.tile([128, 512], bf16, tag="k")
                    nc.vector.tensor_copy(k_sbt[:, :N], k_ps[:, :N])

                    first = gi == 0
                    last = gi == len(groups) - 1
                    for pi in range(cnt):
                        for jh in (0, 1):
                            nc.tensor.matmul(
                                dots[jh],
                                lhsT=k_sbt[:, pi * T + jh * 128:
                                           pi * T + jh * 128 + 128],
                                rhs=q_sbt[:, pi * T:(pi + 1) * T],
                                start=first and pi == 0,
                                stop=last and pi == cnt - 1,
                            )

                # ---- softmax over j (= partitions of dots_T) ----
                attn = [attn_pool.tile([128, T], bf16, tag=f"attn{jc}",
                                       name=f"attn{jc}") for jc in (0, 1)]
                for jc in (0, 1):
                    nc.scalar.activation(attn[jc], dots[jc], AF.Exp)
                s_ps = ps_dots.tile([1, T], f32, tag="dots", name="ssum")
                for jc in (0, 1):
                    nc.tensor.matmul(s_ps, lhsT=ones_pb, rhs=attn[jc],
                                     start=jc == 0, stop=jc == 1)
                r_sb = small_pool.tile([1, T], f32, tag="rsb")
                nc.vector.reciprocal(r_sb, s_ps)
                rb_ps = ps_dots.tile([128, T], f32, tag="dots", name="rbcast")
                nc.tensor.matmul(rb_ps, lhsT=ones_r, rhs=r_sb,
                                 start=True, stop=True)
                for jc in (0, 1):
                    nc.vector.tensor_mul(attn[jc], attn[jc], rb_ps)

                # ---- phase 2: v-conv, attention-average, out-conv, residual ----
                for g in groups:
                    cnt = 1 if g[1] is None else 2
                    N = cnt * T
                    base = g[2]

                    vsb = []
                    for pi in range(cnt):
                        v_ps = ps_work.tile([128, 512], f32, tag="pwork")
                        for jc in (0, 1):
                            nc.tensor.matmul(
                                v_ps[:, jc * HIDE_C:(jc + 1) * HIDE_C],
                                lhsT=x_winb[:, base + pi * T + jc * 128:
                                            base + pi * T + jc * 128 + 128],
                                rhs=wvT,
                                start=True, stop=True,
                            )
                        v_sbt = v_sb_pool.tile([128, 512], bf16, tag="v")
                        nc.vector.tensor_copy(v_sbt, v_ps)
                        vsb.append(v_sbt)

                    g_tiles = []
                    for kc in (0, 1):
                        av = ps_av.tile([128, 512], f32, tag="av", name=f"av{kc}")
                        for pi in range(cnt):
                            for jc in (0, 1):
                                nc.tensor.matmul(
                                    av[:, pi * T:(pi + 1) * T],
                                    lhsT=vsb[pi][:, jc * HIDE_C + kc * 128:
                                                  jc * HIDE_C + kc * 128 + 128],
                                    rhs=attn[jc],
                                    start=jc == 0, stop=jc == 1,
                                )
                        g_t = g_sb_pool.tile([128, 512], bf16, tag=f"g{kc}")
                        nc.scalar.activation(g_t[:, :N], av[:, :N], AF.Gelu,
                                             bias=bv_ap[kc], scale=1.0)
                        g_tiles.append(g_t)

                    o_ps = ps_o_pool.tile([128, 512], f32, tag="ops")
                    for pi in range(cnt):
                        for kc in (0, 1):
                            nc.tensor.matmul(
                                o_ps[:, pi * T:(pi + 1) * T],
                                lhsT=woT[:, kc, :],
                                rhs=g_tiles[kc][:, pi * T:(pi + 1) * T],
                                start=kc == 0, stop=kc == 1,
                            )
                    # residual add (fp32 x) + final gelu, written back IN PLACE
                    # into the x image (these columns are dead after the read)
                    tmp = tmp_pool.tile([128, 512], f32, tag="tmp")
                    nc.vector.tensor_add(tmp[:, :N], o_ps[:, :N], grp_ap(g))
                    nc.scalar.activation(grp_ap(g), tmp[:, :N], AF.Gelu,
                                         bias=bo_ap, scale=1.0)

                # ---- store (x_img now holds the output image) ----
                nc.sync.dma_start(out=out_d.ap()[img], in_=x_img)

    nc.compile()
    return nc


def fold_params(wq, gq, bq, mq, vq, wk, gk, bk, mk, vk,
                wv, gv, bv, mv, vv, wo, bo, go, bbo, mo, vo):
    """Host-side BN/bias folding. Returns transposed folded bf16 weights and
    the packed fp32 per-partition bias columns."""
    import ml_dtypes
    bf16 = ml_dtypes.bfloat16

    aq = gq / np.sqrt(vq + EPS)
    wq_f = (SCALE * aq)[:, None] * wq
    Bq = SCALE * (bq - aq * mq)

    ak = gk / np.sqrt(vk + EPS)
    wk_f = ak[:, None] * wk          # k bias drops (softmax shift invariance)

    av = gv / np.sqrt(vv + EPS)
    wv_f = av[:, None] * wv
    Bv = bv - av * mv                # applied inside the first gelu

    ao = go / np.sqrt(vo + EPS)
    wo_f = ao[:, None] * wo
    Bo = ao * (bo - mo) + bbo        # conv bias + BN fold, inside last gelu

    biases = np.stack([Bq, Bv[:128], Bv[128:], Bo], axis=1).astype(F32)
    return (np.ascontiguousarray(wq_f.T).astype(bf16),
            np.ascontiguousarray(wk_f.T).astype(bf16),
            np.ascontiguousarray(wv_f.T).astype(bf16),
            np.ascontiguousarray(wo_f.T).astype(bf16),
            biases)


_CACHED = {}


def _get_nc(bpc=BPC):
    if bpc not in _CACHED:
        _CACHED[bpc] = build_bass_kernel(bpc)
    return _CACHED[bpc]


def kernel(**inputs):
    from concourse.bass_utils import run_bass_kernel_spmd

    x = np.asarray(inputs["x"], F32)
    wqT, wkT, wvT, woT, biases = fold_params(
        *[np.asarray(inputs[k], F32) for k in
          ("wq", "gq", "bq", "mq", "vq", "wk", "gk", "bk", "mk", "vk",
           "wv", "gv", "bv", "mv", "vv", "wo", "bo", "go", "bbo", "mo", "vo")]
    )

    nc = _get_nc(BPC)
    in_maps = []
    for c in range(NCORES):
        xs = np.ascontiguousarray(
            x[c * BPC:(c + 1) * BPC].reshape(BPC, IN_C, HW))
        in_maps.append({"x": xs, "wqT": wqT, "wkT": wkT, "wvT": wvT,
                        "woT": woT, "biases": biases})

    res = run_bass_kernel_spmd(nc, in_maps, list(range(NCORES)))
    outs = [res.results[c]["out"].reshape(BPC, OUT_C, H, W)
            for c in range(NCORES)]
    return np.concatenate(outs, axis=0)
